# revision 7
# baseline (speedup 1.0000x reference)
"""CTC loss (blank = last class), batch-sharded across 8 NeuronCores.

Algorithm (per core, 32 examples):
  - The CTC forward DP is run in the LINEAR domain as a loop over the
    S = 129 extended-label states; each state's full time series is one
    hardware `tensor_tensor_scan` (state = (u_t + state) * P_t) on the
    vector engine.  Cross-state coupling u_t = a[s-1,t-1] + m*a[s-2,t-1]
    is produced by PE matmuls (identity + per-example masked diagonal)
    reading guard-inclusive shifted slices, so the shift costs nothing.
  - Emission probabilities P[s,t] = exp(logit[y_ext[s], t] - DELTA) are
    gathered with a one-hot matmul over classes; that needs logits in
    [C, T] layout, produced by exp (ScalarE) + DMA x-bar transposes of
    bf16 tiles.  A 66-column one-hot (64 labels, blank, all-ones) also
    yields the per-frame sum(exp) used for the log-sum-exp correction.
  - Dynamic range: f32 can't hold the full 256-step product, so T is
    split in two 128-step chunks with a per-example rescale (anchored
    at the states the loss actually reads) between them.
  - Host side: nll_b = lsesum_b - log(fin[end] + fin[end-1]) - log(r_b),
    then the mean.  All prescale constants cancel exactly.
"""

import numpy as np
import ml_dtypes

B, T, C, L = 256, 256, 512, 64
NCORES = 8
BS = B // NCORES          # 32 examples per core
S = 2 * L + 1             # 129 extended states
BLANK = C - 1
DELTA = 0.957             # exp prescale (centers the linear-domain drift)
TCH = 128                 # time chunk length
BLK = TCH + 1             # state block: guard col + TCH
CHUNK_EX = 4              # examples per load chunk (1 MB of bf16 logits)

_BF16 = ml_dtypes.bfloat16
_ONE = np.float32(1.0).astype(_BF16).view(np.uint16)  # bf16 1.0 bit pattern

_STATE = {}


def _build_nc():
    import concourse.bass as bass
    import concourse.tile as tile
    from concourse import mybir

    f32, bf16 = mybir.dt.float32, mybir.dt.bfloat16
    Add, Mult = mybir.AluOpType.add, mybir.AluOpType.mult
    Exp = mybir.ActivationFunctionType.Exp
    Log = mybir.ActivationFunctionType.Ln
    X = mybir.AxisListType.X

    nc = bass.Bass("TRN2", target_bir_lowering=False, debug=False)
    lg = nc.declare_dram_parameter("logits", [BS, T, C], bf16, isOutput=False)
    oh = nc.declare_dram_parameter("onehot", [128, BS * 4 * 66], bf16, isOutput=False)
    dg = nc.declare_dram_parameter("diag", [BS, 64 * BS], bf16, isOutput=False)
    ey = nc.declare_dram_parameter("eye", [BS, BS], bf16, isOutput=False)
    es = nc.declare_dram_parameter("endsel", [BS, S], f32, isOutput=False)
    out = nc.declare_dram_parameter("out", [BS, S + 2], f32, isOutput=True)

    with tile.TileContext(nc) as tc:
        with (
            tc.tile_pool(name="const", bufs=1) as cpool,
            tc.tile_pool(name="ect", bufs=1) as epool,
            tc.tile_pool(name="pall", bufs=1) as ppool,
            tc.tile_pool(name="io", bufs=2) as iopool,
            tc.tile_pool(name="stg", bufs=4) as spool,
            tc.tile_pool(name="alpha", bufs=1) as apool,
            tc.tile_pool(name="psg", bufs=2, space=bass.MemorySpace.PSUM) as pgpool,
            tc.tile_pool(name="pss", bufs=4, space=bass.MemorySpace.PSUM) as sgpool,
        ):
            # ---- constants ----
            oht = cpool.tile([128, BS * 4 * 66], bf16, tag="oh")
            nc.sync.dma_start(oht[:], oh[:])
            ohr = oht[:].rearrange("p (e c s) -> p e c s", e=BS, c=4)
            dgt = cpool.tile([BS, 64 * BS], bf16, tag="dg")
            nc.sync.dma_start(dgt[:], dg[:])
            dgr = dgt[:].rearrange("p (j b) -> p j b", b=BS)
            eyt = cpool.tile([BS, BS], bf16, tag="ey")
            nc.sync.dma_start(eyt[:], ey[:])
            est = cpool.tile([BS, S], f32, tag="es")
            nc.sync.dma_start(est[:], es[:])
            zeros = cpool.tile([BS, TCH], bf16, tag="z")
            nc.vector.memset(zeros[:], 0.0)
            dbias = cpool.tile([128, 1], f32, tag="db")
            nc.vector.memset(dbias[:], -DELTA)

            # ---- phase 1: exp + transpose to [C, T] ----
            ect = [epool.tile([128, BS * T], bf16, tag=f"ect{i}", name=f"ect{i}")
                   for i in range(4)]
            ectr = [t[:].rearrange("p (e t) -> p e t", t=T) for t in ect]
            for ci in range(BS // CHUNK_EX):
                lgt = iopool.tile([128, CHUNK_EX * 2 * C], bf16, tag="lg")
                nc.sync.dma_start(
                    lgt[:].rearrange("p (e a c) -> p e a c", e=CHUNK_EX, a=2),
                    lg[ci * CHUNK_EX:(ci + 1) * CHUNK_EX].rearrange(
                        "e (a p) c -> p e a c", p=128),
                )
                ett = iopool.tile([128, CHUNK_EX * 2 * C], bf16, tag="et")
                nc.scalar.activation(ett[:], lgt[:], Exp, bias=dbias[:])
                etr = ett[:].rearrange("p (e a c) -> p e a c", e=CHUNK_EX, a=2)
                for x in range(CHUNK_EX):
                    ex = ci * CHUNK_EX + x
                    for ct in range(4):
                        for tt in range(2):
                            nc.sync.dma_start_transpose(
                                ectr[ct][:, ex, tt * 128:(tt + 1) * 128],
                                etr[:, x, tt, ct * 128:(ct + 1) * 128],
                            )

            # ---- phase 2: one-hot gather matmuls -> P_all [BS, 66*T] ----
            pall = ppool.tile([BS, 66 * T], bf16, tag="pall")
            pr = pall[:].rearrange("p (j t) -> p j t", t=T)
            for ex in range(BS):
                ps = pgpool.tile([66, T], f32, tag="g")
                for ct in range(4):
                    nc.tensor.matmul(
                        ps[:], ohr[:, ex, ct, :], ectr[ct][:, ex, :],
                        start=(ct == 0), stop=(ct == 3),
                    )
                stg = spool.tile([66, T], bf16, tag="stg")
                nc.scalar.copy(stg[:], ps[:])
                nc.sync.dma_start(pall[ex:ex + 1, :], stg[:])

            # ---- lse correction: sum_t log(sumexp) ----
            lsT = cpool.tile([BS, 1], f32, tag="ls")
            lgtile = spool.tile([BS, T], f32, tag="lse")
            nc.scalar.activation(lgtile[:], pr[:, 65, :], Log)
            nc.vector.reduce_sum(lsT[:], lgtile[:], axis=X)

            # ---- phase 3: state-loop scans over two time chunks ----
            bndsc = cpool.tile([BS, S], bf16, tag="bndsc")
            fin = cpool.tile([BS, S], f32, tag="fin")
            rT = cpool.tile([BS, 1], f32, tag="r")
            for ch in range(2):
                A = apool.tile([BS, S * BLK], bf16, tag="A")
                Ar = A[:].rearrange("p (s c) -> p s c", c=BLK)
                if ch == 0:
                    nc.vector.memset(Ar[:, :, 0], 0.0)
                    nc.vector.memset(Ar[:, 0:1, 0], 1.0)  # alpha_{-1}[0] = 1
                else:
                    nc.vector.tensor_copy(Ar[:, :, 0], bndsc[:])
                for s in range(S):
                    j = (s - 1) // 2
                    d1 = pr[:, 64 if s % 2 == 0 else j, ch * TCH:(ch + 1) * TCH]
                    if ch == 0:
                        ini = 1.0 if s == 0 else 0.0
                    else:
                        ini = bndsc[:, s:s + 1]
                    if s == 0:
                        d0 = zeros[:]
                    elif s % 2 == 1 and s >= 3:
                        sg = sgpool.tile([BS, TCH], f32, tag="sg")
                        nc.tensor.matmul(sg[:], eyt[:], Ar[:, s - 1, 0:TCH],
                                         start=True, stop=False)
                        nc.tensor.matmul(sg[:], dgr[:, j, :], Ar[:, s - 2, 0:TCH],
                                         start=False, stop=True)
                        d0 = sg[:]
                    else:
                        d0 = Ar[:, s - 1, 0:TCH]
                    nc.vector.tensor_tensor_scan(Ar[:, s, 1:BLK], d0, d1, ini,
                                                 Add, Mult)
                if ch == 0:
                    bnd = cpool.tile([BS, S], f32, tag="bnd")
                    nc.vector.tensor_copy(bnd[:], Ar[:, :, TCH])
                    tmp = cpool.tile([BS, S], f32, tag="tmp")
                    nc.vector.tensor_tensor(tmp[:], bnd[:], est[:], Mult)
                    prt = cpool.tile([BS, 1], f32, tag="pr")
                    nc.vector.reduce_sum(prt[:], tmp[:], axis=X)
                    nc.vector.reciprocal(rT[:], prt[:])
                    nc.scalar.mul(bndsc[:], bnd[:], rT[:])
                else:
                    nc.vector.tensor_copy(fin[:], Ar[:, :, TCH])

            nc.sync.dma_start(out[:, 0:S], fin[:])
            nc.sync.dma_start(out[:, S:S + 1], lsT[:])
            nc.sync.dma_start(out[:, S + 1:S + 2], rT[:])
    return nc


def _host_inputs(logits, labels):
    """Per-core input dicts (host-side prep: bf16 cast, one-hot, masks)."""
    lg_bf = np.ascontiguousarray(logits).astype(_BF16)
    in_maps = []
    for core in range(NCORES):
        sl = slice(core * BS, (core + 1) * BS)
        lab = labels[sl].astype(np.int64)           # [BS, 64]
        ohu = np.zeros((128, BS, 4, 66), np.uint16)
        ohu[:, :, :, 65] = _ONE                     # all-ones col -> sum(exp)
        ohu[127, :, 3, 64] = _ONE                   # blank = 511 = 3*128+127
        e_idx = np.repeat(np.arange(BS), 64)
        j_idx = np.tile(np.arange(64), BS)
        cls = lab.reshape(-1)
        ohu[cls % 128, e_idx, cls // 128, j_idx] = _ONE
        diag = np.zeros((BS, 64, BS), np.float32)
        m = np.zeros((BS, 64), np.float32)
        m[:, 1:] = (lab[:, 1:] != lab[:, :-1]).astype(np.float32)
        diag[np.arange(BS)[:, None], np.arange(64)[None, :],
             np.arange(BS)[:, None]] = m
        endsel = np.zeros((BS, S), np.float32)  # filled by _fill_endsel
        in_maps.append({
            "logits": lg_bf[sl],
            "onehot": ohu.reshape(128, BS * 4 * 66).view(_BF16),
            "diag": diag.reshape(BS, 64 * BS).astype(_BF16),
            "eye": np.eye(BS, dtype=_BF16),
            "endsel": endsel,
        })
    return in_maps


def _fill_endsel(in_maps, label_length):
    for core in range(NCORES):
        sl = slice(core * BS, (core + 1) * BS)
        end = 2 * np.asarray(label_length[sl], np.int64)
        endsel = np.zeros((BS, S), np.float32)
        b = np.arange(BS)
        endsel[b, end] = 1.0
        endsel[b, end - 1] = 1.0
        in_maps[core]["endsel"] = endsel
    return in_maps


def _get_runner():
    """Build the Bass program once and keep a persistently-jitted runner."""
    if "fn" in _STATE:
        return _STATE
    import jax
    from jax.sharding import Mesh, PartitionSpec
    from jax.experimental.shard_map import shard_map
    from concourse import bass2jax, mybir

    bass2jax.install_neuronx_cc_hook()
    nc = _build_nc()

    in_names, out_names, out_avals = [], [], []
    for alloc in nc.m.functions[0].allocations:
        if not isinstance(alloc, mybir.MemoryLocationSet):
            continue
        name = alloc.memorylocations[0].name
        if alloc.kind == "ExternalInput":
            in_names.append(name)
        elif alloc.kind == "ExternalOutput":
            out_names.append(name)
            out_avals.append(jax.core.ShapedArray(
                tuple(alloc.tensor_shape), mybir.dt.np(alloc.dtype)))
    n_params = len(in_names)
    all_names = in_names + out_names
    donate = tuple(range(n_params, n_params + len(out_names)))

    def _body(*args):
        outs = bass2jax._bass_exec_p.bind(
            *args,
            out_avals=tuple(out_avals),
            in_names=tuple(all_names),
            out_names=tuple(out_names),
            lowering_input_output_aliases=(),
            sim_require_finite=True,
            sim_require_nnan=True,
            nc=nc,
        )
        return tuple(outs)

    devices = jax.devices()[:NCORES]
    mesh = Mesh(np.asarray(devices), ("core",))
    specs = (PartitionSpec("core"),) * (n_params + len(out_names))
    fn = jax.jit(
        shard_map(_body, mesh=mesh, in_specs=specs,
                  out_specs=(PartitionSpec("core"),) * len(out_names),
                  check_rep=False),
        donate_argnums=donate, keep_unused=True,
    )
    _STATE.update(nc=nc, fn=fn, in_names=in_names, out_names=out_names,
                  out_avals=out_avals)
    return _STATE


def _run_device(in_maps):
    st = _get_runner()
    concat_in = [
        np.concatenate([in_maps[c][name] for c in range(NCORES)], axis=0)
        for name in st["in_names"]
    ]
    zeros = [np.zeros((NCORES * av.shape[0],) + tuple(av.shape[1:]), av.dtype)
             for av in st["out_avals"]]
    outs = st["fn"](*concat_in, *zeros)
    res = np.asarray(outs[0]).reshape(NCORES, BS, S + 2)
    return res.reshape(B, S + 2)


def _finish_host(res, label_length):
    b = np.arange(B)
    end = 2 * np.asarray(label_length, np.int64)
    fin = res[:, 0:S].astype(np.float64)
    lsesum = res[:, S].astype(np.float64)
    r = res[:, S + 1].astype(np.float64)
    pair = fin[b, end] + fin[b, end - 1]
    nll = lsesum - np.log(pair) - np.log(r)
    return np.float32(np.mean(nll))


def _kernel_numpy_fallback(logits, labels, label_length, logit_length):
    """Stable log-domain reference (host), for non-standard inputs."""
    NEG = np.float32(-1e30)
    lg = np.asarray(logits, np.float32)
    Bq, Tq, Cq = lg.shape
    Lq = labels.shape[1]
    Sq = 2 * Lq + 1
    mx = lg.max(axis=2, keepdims=True)
    logp = lg - (mx + np.log(np.exp(lg - mx).sum(axis=2, keepdims=True)))
    y_ext = np.full((Bq, Sq), Cq - 1, np.int64)
    y_ext[:, 1::2] = labels
    y_m2 = np.full((Bq, Sq), Cq - 1, np.int64)
    y_m2[:, 2:] = y_ext[:, :-2]
    s_idx = np.arange(Sq)
    skip = (s_idx[None, :] >= 2) & (y_ext != Cq - 1) & (y_ext != y_m2)
    emit = np.take_along_axis(
        logp, np.broadcast_to(y_ext[:, None, :], (Bq, Tq, Sq)), 2)
    alpha = np.where(s_idx[None, :] <= 1, emit[:, 0, :], NEG).astype(np.float32)
    tlast = np.asarray(logit_length, np.int64) - 1
    final = np.full((Bq, Sq), NEG, np.float32)
    if np.any(tlast == 0):
        final[tlast == 0] = alpha[tlast == 0]
    for t in range(1, Tq):
        a1 = np.concatenate([np.full((Bq, 1), NEG, np.float32), alpha[:, :-1]], 1)
        a2 = np.concatenate([np.full((Bq, 2), NEG, np.float32), alpha[:, :-2]], 1)
        a2 = np.where(skip, a2, NEG)
        alpha = (np.logaddexp(np.logaddexp(alpha, a1), a2) + emit[:, t, :])
        sel = tlast == t
        if np.any(sel):
            final[sel] = alpha[sel]
    b = np.arange(Bq)
    end = 2 * np.asarray(label_length, np.int64)
    nll = -np.logaddexp(final[b, end], final[b, end - 1])
    return np.float32(np.mean(nll))


def kernel(logits, labels, label_length, logit_length):
    logits = np.asarray(logits)
    labels = np.asarray(labels)
    label_length = np.asarray(label_length)
    logit_length = np.asarray(logit_length)
    if (logits.shape != (B, T, C) or labels.shape != (B, L)
            or np.any(logit_length != T)):
        return _kernel_numpy_fallback(logits, labels, label_length, logit_length)
    try:
        in_maps = _host_inputs(logits, labels)
        _fill_endsel(in_maps, label_length)
        res = _run_device(in_maps)
        val = _finish_host(res, label_length)
        if not np.isfinite(val):
            raise FloatingPointError("non-finite device result")
        return val
    except Exception:
        return _kernel_numpy_fallback(logits, labels, label_length, logit_length)
